# revision 1
# baseline (speedup 1.0000x reference)
"""Multi-head attention block on 8 Trainium2 NeuronCores.

Problem: B=4, N=2048, C=768, H=12, HD=64 (f32).
  qkv = x @ w_qkv + b_qkv ; attn = softmax(q*k^T/8) ; out = (attn@v) @ w_proj + b_proj

Sharding: data-parallel over batch (4) x tensor-parallel over heads (2 groups
of 6 heads). Core c handles batch c//2, head-group c%2. Each core computes a
partial projection output [N, C]; the host sums the two head-group partials
per batch and adds b_proj.

Device kernel layout strategy (per core):
  - x^T tiles produced by PE transpose (needed since matmul contracts over
    partitions).
  - q^T, k^T computed in [head*HD, N] layout (lhsT = w tiles, rhs = x^T).
  - v computed in row layout [N, 6*65] with a ones column appended per head
    (the ones column makes attn@V also produce the softmax denominator).
  - scores^T[keys, q] = k^T-tile.T @ q^T, two heads packed in the PE array
    via row tiling (head even at partitions 0-63, head odd at 64-127).
  - softmax without max subtraction (scores are ~N(0,1); exp is safe in f32).
  - attn@V accumulates over 16 key tiles into psum [65, 512]; row 64 = sums.
  - normalize: reciprocal of sums row, broadcast to 64 partitions via a tiny
    K=1 matmul with a ones vector, then DVE multiply.
  - proj contracts o^T per head (K=64) into the final [128, 768] output tile.
Matmuls run as float32r (full-rate fp32 path, needs free dim >= 256).
"""

import numpy as np

from concourse import bacc, bass, bass_utils, tile
from concourse import mybir

B, N, C, H, HD = 4, 2048, 768, 12, 64
SCALE = HD ** -0.5
P = 128
QC = 512              # q-chunk (free dim per matmul)
NT = N // P           # 16 n-tiles
CT = C // P           # 6 contraction tiles over C
NCH = N // QC         # 4 n-chunks
HPC = 6               # heads per core
CQK = HPC * HD        # 384
VW = 65               # V columns per head incl. ones column
F32 = mybir.dt.float32
F32R = mybir.dt.float32r
BF16 = mybir.dt.bfloat16
EXP = mybir.ActivationFunctionType.Exp

_CACHE = {}


def build_program(mm_dt=BF16):
    MMDT = mm_dt
    nc = bacc.Bacc("TRN2", target_bir_lowering=False, debug=False, num_devices=8)

    wdt = F32
    x_d = nc.dram_tensor("x", [N, C], F32, kind="ExternalInput")
    w_d = nc.dram_tensor("w", [C, 3 * CQK], wdt, kind="ExternalInput")
    wp_d = nc.dram_tensor("wp", [CQK, C], wdt, kind="ExternalInput")
    bqk_d = nc.dram_tensor("bqk", [P, CT], F32, kind="ExternalInput")
    bv_d = nc.dram_tensor("bv", [1, CQK], wdt, kind="ExternalInput")
    ident_d = nc.dram_tensor("ident", [P, P], F32, kind="ExternalInput")
    out_d = nc.dram_tensor("out", [N, C], F32, kind="ExternalOutput")

    with tile.TileContext(nc) as tc, nc.allow_low_precision(
            reason="float32r storage feeds the fast PE fp32 path"):
        with (
            tc.tile_pool(name="const", bufs=1) as cpool,
            tc.tile_pool(name="persist", bufs=1) as pp,
        ):
            ident = cpool.tile([P, P], F32, name="ident", tag="ident")
            nc.sync.dma_start(ident[:], ident_d[:])
            ones_f = cpool.tile([P, QC], F32, name="ones_f", tag="ones_f")
            nc.gpsimd.memset(ones_f[:], 1.0)
            ones = cpool.tile([P, QC], MMDT, name="ones", tag="ones")
            nc.vector.tensor_copy(ones[:], ones_f[:])
            zbias = cpool.tile([P, 1], F32, name="zbias", tag="zbias")
            nc.gpsimd.memset(zbias[:], 0.0)
            bqk = cpool.tile([P, CT], F32, name="bqk", tag="bqk")
            nc.sync.dma_start(bqk[:], bqk_d[:])
            bv = cpool.tile([1, CQK], MMDT, name="bv", tag="bv")
            if True:
                bv_f = cpool.tile([1, CQK], F32, name="bv_f", tag="bv_f")
                nc.sync.dma_start(bv_f[:], bv_d[:])
                nc.vector.tensor_copy(bv[:], bv_f[:])
            else:
                nc.sync.dma_start(bv[:], bv_d[:])

            # persistent SBUF arrays
            w_sb = []
            wp_sb = []
            with tc.tile_pool(name="wstage", bufs=2) as wst:
                for ct in range(CT):
                    t = pp.tile([P, 3 * CQK], MMDT, name=f"w{ct}", tag=f"w{ct}")
                    if True:
                        tf = wst.tile([P, 3 * CQK], F32, name="wf", tag="wf")
                        nc.sync.dma_start(tf[:], w_d[ct * P:(ct + 1) * P, :])
                        nc.vector.tensor_copy(t[:], tf[:])
                    else:
                        nc.sync.dma_start(t[:], w_d[ct * P:(ct + 1) * P, :])
                    w_sb.append(t)
                for h in range(HPC):
                    t = pp.tile([HD, C], MMDT, name=f"wp{h}", tag=f"wp{h}")
                    if True:
                        tf = wst.tile([HD, C], F32, name="wpf", tag="wpf")
                        nc.sync.dma_start(tf[:], wp_d[h * HD:(h + 1) * HD, :])
                        nc.vector.tensor_copy(t[:], tf[:])
                    else:
                        nc.sync.dma_start(t[:], wp_d[h * HD:(h + 1) * HD, :])
                    wp_sb.append(t)
            qT = [pp.tile([P, N], MMDT, name=f"q{i}", tag=f"q{i}") for i in range(3)]
            kT = [pp.tile([P, N], MMDT, name=f"k{i}", tag=f"k{i}") for i in range(3)]
            v_sb = pp.tile([P, NT * HPC * VW], MMDT, name="v", tag="v")
            # ones columns of v (col 64 of each head block)
            v_view = v_sb[:].rearrange("p (t w) -> p t w", w=VW)
            nc.vector.tensor_copy(
                v_view[:, :, HD:HD + 1],
                ones_f[:, 0:NT * HPC].rearrange("p (t w) -> p t w", w=1))
            o_sb = [pp.tile([HD, N], MMDT, name=f"o{h}", tag=f"o{h}") for h in range(HPC)]

            # ---------------- phase 1: x^T, qkv ----------------
            with (
                tc.tile_pool(name="xraw", bufs=2) as xr_pool,
                tc.tile_pool(name="xt", bufs=2) as xt_pool,
                tc.tile_pool(name="p1ps", bufs=2, space="PSUM") as p1ps,
                tc.tile_pool(name="qkps", bufs=2, space="PSUM") as qkps,
            ):
                for j in range(NCH):
                    xtc = [xt_pool.tile([P, QC], MMDT, name=f"xt{ct}", tag=f"xt{ct}")
                           for ct in range(CT)]
                    for ntl in range(QC // P):
                        nt = j * (QC // P) + ntl
                        xr = xr_pool.tile([P, C], F32, name="xr", tag="xr")
                        nc.sync.dma_start(xr[:], x_d[nt * P:(nt + 1) * P, :])
                        for ct in range(CT):
                            tp = p1ps.tile([P, P], F32, name="trps", tag="trps")
                            nc.tensor.transpose(
                                tp[:], xr[:, ct * P:(ct + 1) * P], ident[:])
                            nc.vector.tensor_copy(
                                xtc[ct][:, ntl * P:(ntl + 1) * P], tp[:])
                        # V rows for this n-tile (+ bias via K=1 matmul)
                        vps = p1ps.tile([P, CQK], F32, name="vps", tag="vps")
                        for ct in range(CT):
                            nc.tensor.matmul(
                                vps[:],
                                xtc[ct][:, ntl * P:(ntl + 1) * P],
                                w_sb[ct][:, 2 * CQK:3 * CQK],
                                start=(ct == 0), stop=False)
                        nc.tensor.matmul(
                            vps[:], ones[0:1, 0:P],
                            bv[:], start=False, stop=True)
                        for h in range(HPC):
                            nc.vector.tensor_copy(
                                v_sb[:, (nt * HPC + h) * VW:
                                     (nt * HPC + h) * VW + HD],
                                vps[:, h * HD:(h + 1) * HD])
                    # q^T / k^T for this n-chunk
                    for colt in range(2 * 3):
                        qps = qkps.tile([P, QC], F32, name="qkp", tag="qkp")
                        for ct in range(CT):
                            nc.tensor.matmul(
                                qps[:],
                                w_sb[ct][:, colt * P:(colt + 1) * P],
                                xtc[ct][:, :],
                                start=(ct == 0), stop=(ct == CT - 1))
                        dest = qT[colt] if colt < 3 else kT[colt - 3]
                        nc.vector.tensor_scalar_add(
                            dest[:, j * QC:(j + 1) * QC], qps[:],
                            bqk[:, colt:colt + 1])

            # ---------------- phase 2+3: attention + proj ----------------
            with (
                tc.tile_pool(name="exps", bufs=4) as exp_pool,
                tc.tile_pool(name="recs", bufs=2) as rec_pool,
                tc.tile_pool(name="avsb", bufs=2) as avs_pool,
                tc.tile_pool(name="outsb", bufs=2) as out_pool,
                tc.tile_pool(name="sps", bufs=4, space="PSUM") as s_ps,
                tc.tile_pool(name="avps", bufs=3, space="PSUM") as av_ps,
                tc.tile_pool(name="bcps", bufs=1, space="PSUM") as bc_ps,
            ):
                for j in range(NCH):
                    qsl = slice(j * QC, (j + 1) * QC)
                    for p in range(3):
                        av = [av_ps.tile([P, QC], F32, name="av", tag="av")
                              for _ in range(2)]
                        for kt in range(NT):
                            for par in range(2):
                                rows = slice(par * HD, (par + 1) * HD)
                                sps = s_ps.tile([P, QC], F32, name="s", tag="s")
                                nc.tensor.matmul(
                                    sps[:],
                                    kT[p][rows, kt * P:(kt + 1) * P],
                                    qT[p][rows, qsl],
                                    start=True, stop=True)
                                ex = exp_pool.tile([P, QC], MMDT, name="ex", tag="ex")
                                nc.scalar.activation(
                                    ex[:], sps[:], EXP, bias=zbias[:])
                                h = 2 * p + par
                                nc.tensor.matmul(
                                    av[par][0:VW, :],
                                    v_sb[:, (kt * HPC + h) * VW:
                                             (kt * HPC + h + 1) * VW],
                                    ex[:],
                                    start=(kt == 0), stop=(kt == NT - 1))
                        for par in range(2):
                            h = 2 * p + par
                            rec = rec_pool.tile([P, QC], F32, name="rec", tag="rec")
                            nc.vector.reciprocal(
                                rec[HD:HD + 1, :], av[par][HD:HD + 1, :])
                            bc = bc_ps.tile([P, QC], F32, name="bc", tag="bc")
                            nc.tensor.matmul(
                                bc[0:HD, :],
                                ones_f[HD:HD + 1, 0:HD],
                                rec[HD:HD + 1, :],
                                start=True, stop=True)
                            avs = avs_pool.tile([HD, QC], F32, name="avs",
                                                tag="avs")
                            nc.vector.tensor_copy(avs[:], av[par][0:HD, :])
                            nc.vector.tensor_mul(
                                o_sb[h][:, qsl], avs[:], bc[0:HD, :])
                    # proj for the 4 q-tiles of this chunk
                    for tl in range(QC // P):
                        t = j * (QC // P) + tl
                        osb = out_pool.tile([P, C], F32, name="osb", tag="osb")
                        for n0, nw in ((0, QC), (QC, C - QC)):
                            pps = s_ps.tile([P, QC], F32, name="s", tag="s")
                            for h in range(HPC):
                                nc.tensor.matmul(
                                    pps[:, 0:nw],
                                    o_sb[h][:, t * P:(t + 1) * P],
                                    wp_sb[h][:, n0:n0 + nw],
                                    start=(h == 0), stop=(h == HPC - 1))
                            nc.vector.tensor_copy(osb[:, n0:n0 + nw], pps[:, 0:nw])
                        nc.sync.dma_start(out_d[t * P:(t + 1) * P, :], osb[:])

    nc.compile()
    return nc


def _get_program(mm_dt=F32R):
    key = str(mm_dt)
    if key not in _CACHE:
        _CACHE[key] = build_program(mm_dt)
    return _CACHE[key]


def make_in_maps(x, w_qkv, b_qkv, w_proj, mm_dt=None):
    import ml_dtypes
    wnp = np.float32
    x = np.ascontiguousarray(x, np.float32)
    w_qkv = np.asarray(w_qkv, np.float32)
    b_qkv = np.asarray(b_qkv, np.float32)
    w_proj = np.asarray(w_proj, np.float32)
    ident = np.eye(P, dtype=np.float32)
    in_maps = []
    for c in range(8):
        b, hg = divmod(c, 2)
        hsl = slice(hg * HPC * HD, (hg + 1) * HPC * HD)
        wq = w_qkv[:, 0:C][:, hsl] * SCALE
        wk = w_qkv[:, C:2 * C][:, hsl]
        wv = w_qkv[:, 2 * C:3 * C][:, hsl]
        w_in = np.ascontiguousarray(
            np.concatenate([wq, wk, wv], axis=1).astype(wnp))
        bq = b_qkv[0:C][hsl] * SCALE
        bk = b_qkv[C:2 * C][hsl]
        bvv = b_qkv[2 * C:3 * C][hsl]
        bqk_in = np.ascontiguousarray(
            np.concatenate([bq, bk]).reshape(CT, P).T)
        wp_in = np.ascontiguousarray(w_proj[hsl, :].astype(wnp))
        in_maps.append({
            "x": x[b],
            "w": w_in,
            "wp": wp_in,
            "bqk": bqk_in,
            "bv": np.ascontiguousarray(bvv.reshape(1, CQK).astype(wnp)),
            "ident": ident,
        })
    return in_maps


def run(x, w_qkv, b_qkv, w_proj, b_proj, mm_dt=BF16, **run_kwargs):
    nc = _get_program(mm_dt)
    in_maps = make_in_maps(x, w_qkv, b_qkv, w_proj, mm_dt=mm_dt)
    res = bass_utils.run_bass_kernel_spmd(
        nc, in_maps, core_ids=list(range(8)), **run_kwargs)
    y = np.empty((B, N, C), np.float32)
    for b in range(B):
        y[b] = res.results[2 * b]["out"] + res.results[2 * b + 1]["out"]
    y += np.asarray(b_proj, np.float32)
    return y, res


def kernel(x, w_qkv, b_qkv, w_proj, b_proj):
    y, _ = run(x, w_qkv, b_qkv, w_proj, b_proj)
    return y



# revision 4
# speedup vs baseline: 1.3260x; 1.3260x over previous
"""Multi-head attention block on 8 Trainium2 NeuronCores.

Problem: B=4, N=2048, C=768, H=12, HD=64 (f32).
  qkv = x @ w_qkv + b_qkv ; attn = softmax(q*k^T/8) ; out = (attn@v) @ w_proj + b_proj

Sharding: data-parallel over batch (4) x tensor-parallel over heads (2 groups
of 6 heads). Core c handles batch c//2, head-group c%2. Each core computes a
partial projection output [N, C]; the host sums the two head-group partials
per batch and adds b_proj.

v2 design (vs baseline): the run is scalar-engine(exp)-bound, so everything
is organized to keep ACT saturated with F=1024 exps while the PE keeps up:
  - host pre-casts x / weights to bf16 (halves input DMA, kills device casts)
  - x^T via bf16 PE transposes, 6 per n-tile packed into one PSUM bank,
    evacuated with a single strided DVE copy
  - q^T/k^T in [pair*128, N] layout (head pair p at partitions 2p*64..),
    scores matmuls for the two heads of a pair run CONCURRENTLY on the PE
    via row-tiled K=64 placement (partitions 0-63 / 64-127)
  - exp over [128, 1024] PSUM (2 banks) in one ACT instruction
  - attn@V accumulates [65, 1024] PSUM per head (ones column in V gives the
    softmax denominator in row 64); PSUM budget: 4 banks scores ping-pong +
    4 banks av = 8
  - av evacuated to SBUF f32 immediately (frees banks); normalization is
    emitted one pair LATE so it fills pipeline gaps: denominator reciprocal
    (reciprocal_approx_fast), K=1 ones-matmul broadcast across partitions,
    DVE multiply into the pair-stacked o2 [128, N] bf16
  - out-proj contracts K=128 per head pair (o2 stacked), [128, 768] PSUM
"""

import numpy as np

from concourse import bacc, bass, bass_utils, tile
from concourse import mybir

B, N, C, H, HD = 4, 2048, 768, 12, 64
SCALE = HD ** -0.5
P = 128
NT = N // P           # 16 key/n tiles
CT = C // P           # 6 contraction tiles over C
HPC = 6               # heads per core
NPAIR = 3             # head pairs per core
QC = 1024             # attention q-chunk
NCH = N // QC         # 2 q-chunks
VW = 65               # V columns per head incl. ones column
VWP = 80              # padded per-head V block stride
F32 = mybir.dt.float32
F32R = mybir.dt.float32r
BF16 = mybir.dt.bfloat16
EXP = mybir.ActivationFunctionType.Exp

_CACHE = {}


def build_program(mm_dt=BF16):
    MMDT = mm_dt
    nc = bacc.Bacc("TRN2", target_bir_lowering=False, debug=False, num_devices=8)

    x_d = nc.dram_tensor("x", [N, C], MMDT, kind="ExternalInput")
    wqk_d = nc.dram_tensor("wqk", [C, 2 * NPAIR * P], MMDT, kind="ExternalInput")
    wv_d = nc.dram_tensor("wv", [C, HPC * HD], MMDT, kind="ExternalInput")
    wp_d = nc.dram_tensor("wp", [NPAIR * P, C], MMDT, kind="ExternalInput")
    bqk_d = nc.dram_tensor("bqk", [P, CT], F32, kind="ExternalInput")
    bv_d = nc.dram_tensor("bv", [1, HPC * HD], MMDT, kind="ExternalInput")
    ident_d = nc.dram_tensor("ident", [P, P], MMDT, kind="ExternalInput")
    out_d = nc.dram_tensor("out", [N, C], F32, kind="ExternalOutput")

    with tile.TileContext(nc) as tc, nc.allow_low_precision(
            reason="bf16 matmul pipeline, approx reciprocal for softmax denom"):
        with (
            tc.tile_pool(name="const", bufs=1) as cpool,
            tc.tile_pool(name="persist", bufs=1) as pp,
        ):
            ident = cpool.tile([P, P], MMDT, name="ident", tag="ident")
            nc.sync.dma_start(ident[:], ident_d[:])
            onesb = cpool.tile([1, P], MMDT, name="onesb", tag="onesb")
            nc.gpsimd.memset(onesb[:], 1.0)
            ones65 = cpool.tile([1, HD], F32, name="ones65", tag="ones65")
            nc.gpsimd.memset(ones65[:], 1.0)
            bqk = cpool.tile([P, CT], F32, name="bqk", tag="bqk")
            nc.sync.dma_start(bqk[:], bqk_d[:])
            bv = cpool.tile([1, HPC * HD], MMDT, name="bv", tag="bv")
            nc.sync.dma_start(bv[:], bv_d[:])

            # persistent SBUF arrays
            w_sb = []   # q/k weights: per ct, [128, 6*128] (colt c: pair c q / pair c-3 k)
            wv_sb = []  # v weights: per ct, [128, 384]
            wp_sb = []  # proj weights: per pair, [128, 768]
            for ct in range(CT):
                t = pp.tile([P, 2 * NPAIR * P], MMDT, name=f"w{ct}", tag=f"w{ct}")
                nc.sync.dma_start(t[:], wqk_d[ct * P:(ct + 1) * P, :])
                w_sb.append(t)
                tv = pp.tile([P, HPC * HD], MMDT, name=f"wv{ct}", tag=f"wv{ct}")
                nc.sync.dma_start(tv[:], wv_d[ct * P:(ct + 1) * P, :])
                wv_sb.append(tv)
            for p in range(NPAIR):
                t = pp.tile([P, C], MMDT, name=f"wp{p}", tag=f"wp{p}")
                nc.sync.dma_start(t[:], wp_d[p * P:(p + 1) * P, :])
                wp_sb.append(t)

            xT = pp.tile([P, CT * N], MMDT, name="xT", tag="xT")
            qT = [pp.tile([P, N], MMDT, name=f"q{i}", tag=f"q{i}")
                  for i in range(NPAIR)]
            kT = [pp.tile([P, N], MMDT, name=f"k{i}", tag=f"k{i}")
                  for i in range(NPAIR)]
            v_sb = pp.tile([P, NT * HPC * VWP], MMDT, name="v", tag="v")
            # ones columns: memset everything to 1, V writes leave col 64 = 1
            nc.gpsimd.memset(v_sb[:], 1.0)
            o2_sb = [pp.tile([P, N], MMDT, name=f"o2{p}", tag=f"o2{p}")
                     for p in range(NPAIR)]

            # ---------------- phase A: x^T, q/k/v ----------------
            with (
                tc.tile_pool(name="xraw", bufs=3) as xr_pool,
                tc.tile_pool(name="trps", bufs=2, space="PSUM") as tr_ps,
                tc.tile_pool(name="vps", bufs=2, space="PSUM") as v_ps,
                tc.tile_pool(name="qkps", bufs=2, space="PSUM") as qk_ps,
            ):
                for j in range(4):
                    for ntl in range(4):
                        nt = 4 * j + ntl
                        xr = xr_pool.tile([P, C], MMDT, name="xr", tag="xr")
                        nc.sync.dma_start(xr[:], x_d[nt * P:(nt + 1) * P, :])
                        tp = tr_ps.tile([P, C], MMDT, name="tp", tag="tp")
                        for ct in range(CT):
                            nc.tensor.transpose(
                                tp[:, ct * P:(ct + 1) * P],
                                xr[:, ct * P:(ct + 1) * P], ident[:])
                        # xT[:, ct*N + nt*P : +P] for all ct in one strided copy
                        nc.vector.tensor_copy(
                            xT[:].rearrange("p (c n) -> p c n", n=N)
                                [:, :, nt * P:(nt + 1) * P],
                            tp[:].rearrange("p (c n) -> p c n", n=P))
                        # V rows for this n-tile (+ bias via K=1 matmul)
                        vps = v_ps.tile([P, HPC * HD], F32, name="vps", tag="vps")
                        for ct in range(CT):
                            nc.tensor.matmul(
                                vps[:],
                                xT[:, ct * N + nt * P:ct * N + (nt + 1) * P],
                                wv_sb[ct][:],
                                start=(ct == 0), stop=False)
                        nc.tensor.matmul(
                            vps[:], onesb[0:1, :], bv[:], start=False, stop=True)
                        nc.vector.tensor_copy(
                            v_sb[:].rearrange("p (t w) -> p t w", w=VWP)
                                [:, nt * HPC:(nt + 1) * HPC, 0:HD],
                            vps[:].rearrange("p (h w) -> p h w", w=HD))
                    # q^T / k^T for this n-chunk of 512
                    for colt in range(2 * NPAIR):
                        qps = qk_ps.tile([P, 512], F32, name="qkp", tag="qkp")
                        for ct in range(CT):
                            nc.tensor.matmul(
                                qps[:],
                                w_sb[ct][:, colt * P:(colt + 1) * P],
                                xT[:, ct * N + j * 512:ct * N + (j + 1) * 512],
                                start=(ct == 0), stop=(ct == CT - 1))
                        dest = qT[colt] if colt < NPAIR else kT[colt - NPAIR]
                        nc.vector.tensor_scalar_add(
                            dest[:, j * 512:(j + 1) * 512], qps[:],
                            bqk[:, colt:colt + 1])

            # ---------------- phase B: attention ----------------
            with (
                tc.tile_pool(name="sps", bufs=2, space="PSUM") as s_ps,
                tc.tile_pool(name="avps", bufs=2, space="PSUM") as av_ps,
                tc.tile_pool(name="exsb", bufs=3) as ex_pool,
                tc.tile_pool(name="o2u", bufs=4) as o2u_pool,
                tc.tile_pool(name="rec", bufs=2) as rec_pool,
            ):
                def attn_body(c, p):
                    """kt loop for q-chunk c, head pair p. Returns o2u tiles."""
                    qsl = slice(c * QC, (c + 1) * QC)
                    av = [av_ps.tile([VW, QC], F32, name="av", tag="av")
                          for _ in range(2)]
                    for kt in range(NT):
                        for par in range(2):
                            h = 2 * p + par
                            rows = slice(par * HD, (par + 1) * HD)
                            sps = s_ps.tile([P, QC], F32, name="s", tag="s")
                            for half in range(2):
                                fsl = slice(half * 512, (half + 1) * 512)
                                nc.tensor.matmul(
                                    sps[:, fsl],
                                    kT[p][rows, kt * P:(kt + 1) * P],
                                    qT[p][rows, c * QC + half * 512:
                                          c * QC + (half + 1) * 512],
                                    start=True, stop=True)
                            ex = ex_pool.tile([P, QC], MMDT, name="ex", tag="ex")
                            nc.scalar.activation(ex[:], sps[:], EXP)
                            vcol = (kt * HPC + h) * VWP
                            for half in range(2):
                                fsl = slice(half * 512, (half + 1) * 512)
                                nc.tensor.matmul(
                                    av[par][:, fsl],
                                    v_sb[:, vcol:vcol + VW],
                                    ex[:, fsl],
                                    start=(kt == 0), stop=(kt == NT - 1))
                    o2u = []
                    for par in range(2):
                        t = o2u_pool.tile([VW, QC], F32, name="o2u", tag="o2u")
                        nc.vector.tensor_copy(t[:], av[par][:])
                        o2u.append(t)
                    return o2u

                def normalize(c, p, o2u):
                    """Softmax-normalize o2u into o2_sb (pair-stacked bf16)."""
                    qsl = slice(c * QC, (c + 1) * QC)
                    for par in range(2):
                        # reciprocal_approx_fast breaks at base partition 64:
                        # move the denominator row to partition 0 first.
                        den = rec_pool.tile([1, QC], F32, name="den", tag="den")
                        nc.vector.tensor_copy(den[:], o2u[par][HD:VW, :])
                        rec = rec_pool.tile([1, QC], F32, name="rec", tag="rec")
                        nc.vector.reciprocal_approx_fast(rec[:], den[:])
                        bc = s_ps.tile([HD, QC], F32, name="bc", tag="s")
                        for half in range(2):
                            fsl = slice(half * 512, (half + 1) * 512)
                            nc.tensor.matmul(
                                bc[:, fsl],
                                ones65[:],
                                rec[0:1, fsl],
                                start=True, stop=True)
                        nc.vector.tensor_tensor(
                            o2_sb[p][par * HD:(par + 1) * HD, qsl],
                            o2u[par][0:HD, :], bc[:],
                            op=mybir.AluOpType.mult)

                pending = None
                for c in range(NCH):
                    for p in range(NPAIR):
                        o2u = attn_body(c, p)
                        if pending is not None:
                            normalize(*pending)
                        pending = (c, p, o2u)
                assert pending is not None
                normalize(*pending)

            # ---------------- phase C: out-projection ----------------
            with (
                tc.tile_pool(name="pps", bufs=2, space="PSUM") as p_ps,
                tc.tile_pool(name="outsb", bufs=3) as out_pool,
            ):
                for t in range(NT):
                    pps = p_ps.tile([P, C], F32, name="pps", tag="pps")
                    for n0, nw in ((0, 512), (512, C - 512)):
                        for p in range(NPAIR):
                            nc.tensor.matmul(
                                pps[:, n0:n0 + nw],
                                o2_sb[p][:, t * P:(t + 1) * P],
                                wp_sb[p][:, n0:n0 + nw],
                                start=(p == 0), stop=(p == NPAIR - 1))
                    osb = out_pool.tile([P, C], F32, name="osb", tag="osb")
                    nc.vector.tensor_copy(osb[:], pps[:])
                    nc.sync.dma_start(out_d[t * P:(t + 1) * P, :], osb[:])

    nc.compile()
    return nc


def _get_program(mm_dt=BF16):
    key = str(mm_dt)
    if key not in _CACHE:
        _CACHE[key] = build_program(mm_dt)
    return _CACHE[key]


def make_in_maps(x, w_qkv, b_qkv, w_proj):
    import ml_dtypes
    bf = ml_dtypes.bfloat16
    x = np.asarray(x, np.float32)
    w_qkv = np.asarray(w_qkv, np.float32)
    b_qkv = np.asarray(b_qkv, np.float32)
    w_proj = np.asarray(w_proj, np.float32)
    ident = np.eye(P, dtype=np.float32).astype(bf)
    in_maps = []
    for c in range(8):
        b, hg = divmod(c, 2)
        hsl = slice(hg * HPC * HD, (hg + 1) * HPC * HD)
        wq = w_qkv[:, 0:C][:, hsl] * SCALE
        wk = w_qkv[:, C:2 * C][:, hsl]
        wv = w_qkv[:, 2 * C:3 * C][:, hsl]
        wqk_in = np.ascontiguousarray(
            np.concatenate([wq, wk], axis=1).astype(bf))
        bq = b_qkv[0:C][hsl] * SCALE
        bk = b_qkv[C:2 * C][hsl]
        bvv = b_qkv[2 * C:3 * C][hsl]
        bqk_in = np.ascontiguousarray(
            np.concatenate([bq, bk]).reshape(CT, P).T)
        wp_in = np.ascontiguousarray(w_proj[hsl, :].astype(bf))
        in_maps.append({
            "x": np.ascontiguousarray(x[b].astype(bf)),
            "wqk": wqk_in,
            "wv": np.ascontiguousarray(wv.astype(bf)),
            "wp": wp_in,
            "bqk": bqk_in,
            "bv": np.ascontiguousarray(bvv.reshape(1, HPC * HD).astype(bf)),
            "ident": ident,
        })
    return in_maps


def run(x, w_qkv, b_qkv, w_proj, b_proj, mm_dt=BF16, **run_kwargs):
    nc = _get_program(mm_dt)
    in_maps = make_in_maps(x, w_qkv, b_qkv, w_proj)
    res = bass_utils.run_bass_kernel_spmd(
        nc, in_maps, core_ids=list(range(8)), **run_kwargs)
    y = np.empty((B, N, C), np.float32)
    for b in range(B):
        y[b] = res.results[2 * b]["out"] + res.results[2 * b + 1]["out"]
    y += np.asarray(b_proj, np.float32)
    return y, res


def kernel(x, w_qkv, b_qkv, w_proj, b_proj):
    y, _ = run(x, w_qkv, b_qkv, w_proj, b_proj)
    return y


# revision 6
# speedup vs baseline: 1.3300x; 1.0030x over previous
"""Multi-head attention block on 8 Trainium2 NeuronCores.

Problem: B=4, N=2048, C=768, H=12, HD=64 (f32).
  qkv = x @ w_qkv + b_qkv ; attn = softmax(q*k^T/8) ; out = (attn@v) @ w_proj + b_proj

Sharding: data-parallel over batch (4) x tensor-parallel over heads (2 groups
of 6 heads). Core c handles batch c//2, head-group c%2. Each core computes a
partial projection output [N, C]; the host sums the two head-group partials
per batch and adds b_proj.

v3 design. The run is scalar-engine(exp)-bound; everything is organized to
keep ACT saturated with F=1024 exps while the PE keeps pace:
  - host pre-casts x / weights to bf16 (halves input DMA, kills device casts)
  - x^T produced by 6 xbar DMA-transposes straight from DRAM (no PE
    transposes, no staging copies)
  - q^T/k^T in [pair*128, N] layout (head pair p at partitions 2p*64..);
    the two heads of a pair are emitted adjacently so their K=64 row-tiled
    scores matmuls run concurrently on the PE (partitions 0-63 / 64-127)
  - exp over [128, 1024] PSUM (2 banks) in one ACT instruction
  - attn@V accumulates [65, 1024] PSUM per head (ones column in V gives the
    softmax denominator in row 64); PSUM budget: 4 banks scores ping-pong +
    4 banks av = 8
  - at pair end av is evacuated to SBUF f32 on scalar+vector in parallel
    (frees the av banks fast); normalization trails one pair behind:
    denominator reciprocal (reciprocal_approx_fast at partition 0), K=1
    ones-matmul broadcast, DVE multiply into pair-stacked o2 [128, N] bf16
  - out-projection contracts K=128 per head pair (o2 stacked), [128, 768]
    PSUM, 4 output tiles batched per DMA
"""

import numpy as np

from concourse import bacc, bass, bass_utils, tile
from concourse import mybir

B, N, C, H, HD = 4, 2048, 768, 12, 64
SCALE = HD ** -0.5
P = 128
NT = N // P           # 16 key/n tiles
CT = C // P           # 6 contraction tiles over C
HPC = 6               # heads per core
NPAIR = 3             # head pairs per core
QC = 1024             # attention q-chunk
NCH = N // QC         # 2 q-chunks
VW = 65               # V columns per head incl. ones column
VWP = 80              # padded per-head V block stride
F32 = mybir.dt.float32
F32R = mybir.dt.float32r
BF16 = mybir.dt.bfloat16
EXP = mybir.ActivationFunctionType.Exp

_CACHE = {}


def build_program(mm_dt=BF16):
    MMDT = mm_dt
    nc = bacc.Bacc("TRN2", target_bir_lowering=False, debug=False, num_devices=8)

    x_d = nc.dram_tensor("x", [N, C], MMDT, kind="ExternalInput")
    wqk_d = nc.dram_tensor("wqk", [C, 2 * NPAIR * P], MMDT, kind="ExternalInput")
    wv_d = nc.dram_tensor("wv", [C, HPC * HD], MMDT, kind="ExternalInput")
    wp_d = nc.dram_tensor("wp", [NPAIR * P, C], MMDT, kind="ExternalInput")
    bqk_d = nc.dram_tensor("bqk", [P, CT], F32, kind="ExternalInput")
    bv_d = nc.dram_tensor("bv", [1, HPC * HD], MMDT, kind="ExternalInput")
    out_d = nc.dram_tensor("out", [N, C], F32, kind="ExternalOutput")

    with tile.TileContext(nc) as tc, nc.allow_low_precision(
            reason="bf16 matmul pipeline, approx reciprocal for softmax denom"):
        with (
            tc.tile_pool(name="const", bufs=1) as cpool,
            tc.tile_pool(name="persist", bufs=1) as pp,
        ):
            # x^T first: everything downstream depends on it
            xT = [pp.tile([P, N], MMDT, name=f"xT{ct}", tag=f"xT{ct}")
                  for ct in range(CT)]
            for ct in range(CT):
                nc.sync.dma_start(xT[ct][:], x_d[:, ct * P:(ct + 1) * P],
                                  transpose=True)
            wv_sb = []
            for ct in range(CT):
                tv = pp.tile([P, HPC * HD], MMDT, name=f"wv{ct}", tag=f"wv{ct}")
                nc.sync.dma_start(tv[:], wv_d[ct * P:(ct + 1) * P, :])
                wv_sb.append(tv)
            bv = cpool.tile([1, HPC * HD], MMDT, name="bv", tag="bv")
            nc.sync.dma_start(bv[:], bv_d[:])
            w_sb = []
            for ct in range(CT):
                t = pp.tile([P, 2 * NPAIR * P], MMDT, name=f"w{ct}", tag=f"w{ct}")
                nc.sync.dma_start(t[:], wqk_d[ct * P:(ct + 1) * P, :])
                w_sb.append(t)
            bqk = cpool.tile([P, CT], F32, name="bqk", tag="bqk")
            nc.sync.dma_start(bqk[:], bqk_d[:])
            wp_sb = []
            for p in range(NPAIR):
                t = pp.tile([P, C], MMDT, name=f"wp{p}", tag=f"wp{p}")
                nc.sync.dma_start(t[:], wp_d[p * P:(p + 1) * P, :])
                wp_sb.append(t)

            onesb = cpool.tile([1, P], MMDT, name="onesb", tag="onesb")
            nc.gpsimd.memset(onesb[:], 1.0)
            ones65 = cpool.tile([1, HD], F32, name="ones65", tag="ones65")
            nc.gpsimd.memset(ones65[:], 1.0)

            qT = [pp.tile([P, N], MMDT, name=f"q{i}", tag=f"q{i}")
                  for i in range(NPAIR)]
            kT = [pp.tile([P, N], MMDT, name=f"k{i}", tag=f"k{i}")
                  for i in range(NPAIR)]
            v_sb = pp.tile([P, NT * HPC * VWP], MMDT, name="v", tag="v")
            # ones columns: memset everything to 1, V writes leave col 64 = 1
            nc.gpsimd.memset(v_sb[:], 1.0)
            o2_sb = [pp.tile([P, N], MMDT, name=f"o2{p}", tag=f"o2{p}")
                     for p in range(NPAIR)]

            # ---------------- phase A: q/k/v projections ----------------
            with (
                tc.tile_pool(name="vps", bufs=2, space="PSUM") as v_ps,
                tc.tile_pool(name="qkps", bufs=2, space="PSUM") as qk_ps,
            ):
                for j in range(4):
                    for ntl in range(4):
                        nt = 4 * j + ntl
                        # V rows for this n-tile (+ bias via K=1 matmul)
                        vps = v_ps.tile([P, HPC * HD], F32, name="vps", tag="vps")
                        for ct in range(CT):
                            nc.tensor.matmul(
                                vps[:],
                                xT[ct][:, nt * P:(nt + 1) * P],
                                wv_sb[ct][:],
                                start=(ct == 0), stop=False)
                        nc.tensor.matmul(
                            vps[:], onesb[0:1, :], bv[:], start=False, stop=True)
                        nc.vector.tensor_copy(
                            v_sb[:].rearrange("p (t w) -> p t w", w=VWP)
                                [:, nt * HPC:(nt + 1) * HPC, 0:HD],
                            vps[:].rearrange("p (h w) -> p h w", w=HD))
                    # q^T / k^T for this n-chunk of 512
                    for colt in range(2 * NPAIR):
                        qps = qk_ps.tile([P, 512], F32, name="qkp", tag="qkp")
                        for ct in range(CT):
                            nc.tensor.matmul(
                                qps[:],
                                w_sb[ct][:, colt * P:(colt + 1) * P],
                                xT[ct][:, j * 512:(j + 1) * 512],
                                start=(ct == 0), stop=(ct == CT - 1))
                        dest = qT[colt] if colt < NPAIR else kT[colt - NPAIR]
                        nc.vector.tensor_scalar_add(
                            dest[:, j * 512:(j + 1) * 512], qps[:],
                            bqk[:, colt:colt + 1])

            # ---------------- phase B: attention ----------------
            with (
                tc.tile_pool(name="sps", bufs=2, space="PSUM") as s_ps,
                tc.tile_pool(name="avps", bufs=2, space="PSUM") as av_ps,
                tc.tile_pool(name="exsb", bufs=3) as ex_pool,
                tc.tile_pool(name="o2u", bufs=4) as o2u_pool,
                tc.tile_pool(name="rec", bufs=4) as rec_pool,
            ):
                def attn_body(c, p):
                    """kt loop for q-chunk c, head pair p. Returns state for
                    the trailing normalization."""
                    av = [av_ps.tile([VW, QC], F32, name="av", tag="av")
                          for _ in range(2)]
                    for kt in range(NT):
                        sps = [s_ps.tile([P, QC], F32, name="s", tag="s")
                               for _ in range(2)]
                        # both heads' scores adjacent: row groups 0-63/64-127
                        # run concurrently on the PE
                        for half in range(2):
                            for par in range(2):
                                rows = slice(par * HD, (par + 1) * HD)
                                fsl = slice(half * 512, (half + 1) * 512)
                                nc.tensor.matmul(
                                    sps[par][:, fsl],
                                    kT[p][rows, kt * P:(kt + 1) * P],
                                    qT[p][rows, c * QC + half * 512:
                                          c * QC + (half + 1) * 512],
                                    start=True, stop=True)
                        exs = []
                        for par in range(2):
                            ex = ex_pool.tile([P, QC], MMDT, name="ex", tag="ex")
                            nc.scalar.activation(ex[:], sps[par][:], EXP)
                            exs.append(ex)
                        for par in range(2):
                            h = 2 * p + par
                            vcol = (kt * HPC + h) * VWP
                            for half in range(2):
                                fsl = slice(half * 512, (half + 1) * 512)
                                nc.tensor.matmul(
                                    av[par][:, fsl],
                                    v_sb[:, vcol:vcol + VW],
                                    exs[par][:, fsl],
                                    start=(kt == 0), stop=(kt == NT - 1))
                    # evacuate av fast on two engines; denominator row straight
                    # from PSUM so the normalize chain starts immediately
                    o2u, dens = [], []
                    for par in range(2):
                        den = rec_pool.tile([1, QC], F32, name="den", tag="den")
                        nc.vector.tensor_copy(den[:], av[par][HD:VW, :])
                        dens.append(den)
                        t = o2u_pool.tile([HD, QC], F32, name="o2u", tag="o2u")
                        if par == 0:
                            nc.scalar.copy(t[:], av[par][0:HD, :])
                        else:
                            nc.vector.tensor_copy(t[:], av[par][0:HD, :])
                        o2u.append(t)
                    return o2u, dens

                def normalize(c, p, o2u, dens):
                    """Softmax-normalize o2u into o2_sb (pair-stacked bf16)."""
                    qsl = slice(c * QC, (c + 1) * QC)
                    for par in range(2):
                        rec = rec_pool.tile([1, QC], F32, name="rec", tag="rec")
                        nc.vector.reciprocal_approx_fast(rec[:], dens[par][:])
                        bc = s_ps.tile([HD, QC], F32, name="bc", tag="s")
                        for half in range(2):
                            fsl = slice(half * 512, (half + 1) * 512)
                            nc.tensor.matmul(
                                bc[:, fsl],
                                ones65[:],
                                rec[0:1, fsl],
                                start=True, stop=True)
                        nc.vector.tensor_tensor(
                            o2_sb[p][par * HD:(par + 1) * HD, qsl],
                            o2u[par][:], bc[:],
                            op=mybir.AluOpType.mult)

                pending = None
                for c in range(NCH):
                    for p in range(NPAIR):
                        state = attn_body(c, p)
                        if pending is not None:
                            normalize(*pending)
                        pending = (c, p, *state)
                assert pending is not None
                normalize(*pending)

            # ---------------- phase C: out-projection ----------------
            with (
                tc.tile_pool(name="pps", bufs=2, space="PSUM") as p_ps,
                tc.tile_pool(name="outsb", bufs=2) as out_pool,
            ):
                for j in range(4):
                    out4 = out_pool.tile([P, 4 * C], F32, name="out4",
                                         tag="out4")
                    for tl in range(4):
                        t = 4 * j + tl
                        pps = p_ps.tile([P, C], F32, name="pps", tag="pps")
                        for n0, nw in ((0, 512), (512, C - 512)):
                            for p in range(NPAIR):
                                nc.tensor.matmul(
                                    pps[:, n0:n0 + nw],
                                    o2_sb[p][:, t * P:(t + 1) * P],
                                    wp_sb[p][:, n0:n0 + nw],
                                    start=(p == 0), stop=(p == NPAIR - 1))
                        nc.vector.tensor_copy(
                            out4[:, tl * C:(tl + 1) * C], pps[:])
                    nc.sync.dma_start(
                        out_d[j * 512:(j + 1) * 512, :]
                        .rearrange("(a p) c -> p a c", p=P),
                        out4[:].rearrange("p (a c) -> p a c", c=C))

    nc.compile()
    return nc


def _get_program(mm_dt=BF16):
    key = str(mm_dt)
    if key not in _CACHE:
        _CACHE[key] = build_program(mm_dt)
    return _CACHE[key]


def make_in_maps(x, w_qkv, b_qkv, w_proj):
    import ml_dtypes
    bf = ml_dtypes.bfloat16
    x = np.asarray(x, np.float32)
    w_qkv = np.asarray(w_qkv, np.float32)
    b_qkv = np.asarray(b_qkv, np.float32)
    w_proj = np.asarray(w_proj, np.float32)
    in_maps = []
    for c in range(8):
        b, hg = divmod(c, 2)
        hsl = slice(hg * HPC * HD, (hg + 1) * HPC * HD)
        wq = w_qkv[:, 0:C][:, hsl] * SCALE
        wk = w_qkv[:, C:2 * C][:, hsl]
        wv = w_qkv[:, 2 * C:3 * C][:, hsl]
        wqk_in = np.ascontiguousarray(
            np.concatenate([wq, wk], axis=1).astype(bf))
        bq = b_qkv[0:C][hsl] * SCALE
        bk = b_qkv[C:2 * C][hsl]
        bvv = b_qkv[2 * C:3 * C][hsl]
        bqk_in = np.ascontiguousarray(
            np.concatenate([bq, bk]).reshape(CT, P).T)
        wp_in = np.ascontiguousarray(w_proj[hsl, :].astype(bf))
        in_maps.append({
            "x": np.ascontiguousarray(x[b].astype(bf)),
            "wqk": wqk_in,
            "wv": np.ascontiguousarray(wv.astype(bf)),
            "wp": wp_in,
            "bqk": bqk_in,
            "bv": np.ascontiguousarray(bvv.reshape(1, HPC * HD).astype(bf)),
        })
    return in_maps


def run(x, w_qkv, b_qkv, w_proj, b_proj, mm_dt=BF16, **run_kwargs):
    nc = _get_program(mm_dt)
    in_maps = make_in_maps(x, w_qkv, b_qkv, w_proj)
    res = bass_utils.run_bass_kernel_spmd(
        nc, in_maps, core_ids=list(range(8)), **run_kwargs)
    y = np.empty((B, N, C), np.float32)
    for b in range(B):
        y[b] = res.results[2 * b]["out"] + res.results[2 * b + 1]["out"]
    y += np.asarray(b_proj, np.float32)
    return y, res


def kernel(x, w_qkv, b_qkv, w_proj, b_proj):
    y, _ = run(x, w_qkv, b_qkv, w_proj, b_proj)
    return y
